# revision 3
# baseline (speedup 1.0000x reference)
"""Patch-QGAN quantum generator kernel for Trainium2 (8 NeuronCores, SPMD).

Math: the reference evolves |0..0> through an RY embedding layer (angles x/2),
then Q_DEPTH=6 blocks of [per-generator RY layer + CZ-chain sign flip], then
returns probs[..., :256] / sum(probs) normalized by its per-(b,g) max.

Two structural facts make this a matmul problem:
  1. All gates are real; the embedding produces a rank-1 Kronecker vector
     v0(b) = kron_w [cos(x_bw/2), sin(x_bw/2)]  (wire 0 = MSB).
     The remaining 6 blocks depend only on q_params, so they fold into a
     fixed orthogonal matrix M_g per generator: state(b,g) = M_g @ v0(b).
  2. The sum-normalization cancels against the max-normalization:
     (p/S)/max(p/S) == p/max(p).  So only rows 0..255 of M_g are needed.

Per core (batch sharded 8 ways, 512 rows each):
  x -> cos/sin (ScalarE Sin LUT) -> Kronecker doubling (VectorE) -> v0
  -> PE transpose -> v0T -> fp32 matmul vs W = [M_g[:256,:].T]_g (1024x4096)
  -> square (ScalarE) -> per-256-chunk max + reciprocal (VectorE)
  -> scale (ScalarE) -> DMA out.
W is precomputed on host from q_params (tiny: 16x60) in float64.
"""

import os
import sys
import tempfile

import numpy as np

sys.path.insert(0, "/opt/trn_rl_repo")

import concourse.bass as bass
import concourse.tile as tile
from concourse import bacc, mybir
from concourse import bass_utils

N_QUBITS = 10
DIM = 1 << N_QUBITS           # 1024
PATCH = 256
G = 16
Q_DEPTH = 6
B = 4096
N_CORES = 8
B_LOC = B // N_CORES          # 512
BT = B_LOC // 128             # 4 batch tiles per core
KT = DIM // 128               # 8 contraction tiles
CB = (G * PATCH) // 512       # 8 column blocks of 512 (= 2 generators each)

F32 = mybir.dt.float32
F32R = mybir.dt.float32r


def _cz_sign():
    idx = np.arange(DIM)
    shifts = np.arange(N_QUBITS - 1, -1, -1)
    bits = (idx[:, None] >> shifts[None, :]) & 1
    pairs = bits[:, :-1] & bits[:, 1:]
    return np.where(pairs.sum(-1) % 2 == 1, -1.0, 1.0)


def _build_W(q_params: np.ndarray) -> np.ndarray:
    """Rows 0..255 of M_g = D K_5 D K_4 ... D K_0, stacked as (1024, G*256).

    Computed by right-multiplying basis rows S = I[:256] through the chain:
    S @ D scales columns by the CZ sign; S @ K_d applies kron_w RY(-theta_w)
    to each row (RY(t)^T = RY(-t))."""
    w = q_params.reshape(G, Q_DEPTH, N_QUBITS).astype(np.float64)
    sign = _cz_sign()
    S = np.zeros((G, PATCH, DIM))
    S[:, np.arange(PATCH), np.arange(PATCH)] = 1.0
    for d in range(Q_DEPTH - 1, -1, -1):
        S = S * sign[None, None, :]
        ang = -w[:, d, :]
        for wi in range(N_QUBITS):
            half = ang[:, wi] * 0.5
            c = np.cos(half)[:, None, None, None]
            s = np.sin(half)[:, None, None, None]
            St = S.reshape(G, PATCH, 1 << wi, 2, 1 << (N_QUBITS - wi - 1))
            s0, s1 = St[:, :, :, 0, :], St[:, :, :, 1, :]
            S = np.stack((c * s0 - s * s1, s * s0 + c * s1), axis=3)
            S = S.reshape(G, PATCH, DIM)
    # (G, 256, 1024) -> (1024, G*256), column index = g*256 + j
    Wall = np.ascontiguousarray(S.transpose(2, 0, 1).reshape(DIM, G * PATCH))
    return Wall.astype(np.float32)


def _build_nc():
    nc = bacc.Bacc("TRN2", target_bir_lowering=False, debug=False,
                   num_devices=N_CORES)
    x_d = nc.dram_tensor("x", [B_LOC, N_QUBITS], F32, kind="ExternalInput").ap()
    w_d = nc.dram_tensor("w", [DIM, G * PATCH], F32, kind="ExternalInput").ap()
    id_d = nc.dram_tensor("ident", [128, 128], F32, kind="ExternalInput").ap()
    out_d = nc.dram_tensor("out", [B_LOC, G * PATCH], F32,
                           kind="ExternalOutput").ap()

    with tile.TileContext(nc) as tc:
        _body(nc, tc, x_d, w_d, id_d, out_d)
    nc.compile()
    return nc


def _body(nc, tc, x_d, w_d, id_d, out_d):
    from contextlib import ExitStack
    ctx = ExitStack()
    with ctx:
        const = ctx.enter_context(tc.tile_pool(name="const", bufs=1))
        vpool = ctx.enter_context(tc.tile_pool(name="vpool", bufs=2))
        sqp = ctx.enter_context(tc.tile_pool(name="sqp", bufs=4))
        outp = ctx.enter_context(tc.tile_pool(name="outp", bufs=4))
        redp = ctx.enter_context(tc.tile_pool(name="redp", bufs=8))
        psmm = ctx.enter_context(tc.tile_pool(name="psmm", bufs=4, space="PSUM"))
        pstr = ctx.enter_context(tc.tile_pool(name="pstr", bufs=4, space="PSUM"))

        ident = const.tile([128, 128], F32, name="ident_t")
        nc.sync.dma_start(ident[:], id_d[:, :])
        halfpi = const.tile([128, 1], F32, name="halfpi")
        nc.gpsimd.memset(halfpi[:], float(np.float32(np.pi / 2)))

        # Replicated weights: 8 k-tiles of [128, 4096], loaded column-block
        # major so matmuls on early column blocks can start before the whole
        # matrix has arrived.
        w_t = [const.tile([128, G * PATCH], F32R, name=f"w{kt}")
               for kt in range(KT)]
        for cb in range(CB):
            for kt in range(KT):
                nc.sync.dma_start(
                    w_t[kt][:, cb * 512:(cb + 1) * 512],
                    w_d[kt * 128:(kt + 1) * 128,
                        cb * 512:(cb + 1) * 512].bitcast(F32R))

        # v0T: [k, b] layout, 8 tiles of [128, 512]
        v0T = [const.tile([128, B_LOC], F32R, name=f"v0T{kt}")
               for kt in range(KT)]

        for bt in range(BT):
            x_t = vpool.tile([128, N_QUBITS], F32, name=f"x{bt}", tag="x")
            nc.sync.dma_start(x_t[:], x_d[bt * 128:(bt + 1) * 128, :])
            # cs: cols 0..9 = cos(x_w/2), cols 10..19 = sin(x_w/2)
            cs = vpool.tile([128, 2 * N_QUBITS], F32, name=f"cs{bt}", tag="cs")
            nc.scalar.activation(cs[:, N_QUBITS:], x_t[:],
                                 mybir.ActivationFunctionType.Sin,
                                 bias=0.0, scale=0.5)
            nc.scalar.activation(cs[:, :N_QUBITS], x_t[:],
                                 mybir.ActivationFunctionType.Sin,
                                 bias=halfpi[:], scale=0.5)

            # Kronecker doubling: vA/vB ping-pong, wire 9 innermost first.
            vA = vpool.tile([128, DIM], F32, name=f"vA{bt}", tag="vA")
            vB = vpool.tile([128, DIM], F32, name=f"vB{bt}", tag="vB")
            nc.vector.tensor_copy(vA[:, 0:1], cs[:, 9:10])
            nc.vector.tensor_copy(vA[:, 1:2], cs[:, 19:20])
            cur, nxt = vA, vB
            L = 2
            for wi in range(N_QUBITS - 2, -1, -1):
                nc.vector.tensor_scalar_mul(nxt[:, 0:L], cur[:, 0:L],
                                            cs[:, wi:wi + 1])
                nc.vector.tensor_scalar_mul(nxt[:, L:2 * L], cur[:, 0:L],
                                            cs[:, N_QUBITS + wi:N_QUBITS + wi + 1])
                cur, nxt = nxt, cur
                L *= 2
            assert L == DIM

            # Transpose [b,k] -> [k,b] via PE, one 128x128 block per k-tile.
            for kt in range(KT):
                trp = pstr.tile([128, 128], F32, name=f"tr{bt}_{kt}", tag="tr")
                nc.tensor.transpose(trp[:], cur[:, kt * 128:(kt + 1) * 128],
                                    ident[:])
                # fp32 PSUM -> float32r SBUF (verifier requires fp32r-typed
                # producers for fp32r matmul operands; DVE does the cast)
                nc.vector.tensor_copy(
                    v0T[kt][:, bt * 128:(bt + 1) * 128], trp[:])

        # Main pipeline: per (cb, bt): 8 accumulating matmuls -> square ->
        # per-generator max -> reciprocal -> scale -> DMA out.
        for cb in range(CB):
            for bt in range(BT):
                pmm = psmm.tile([128, 512], F32, name=f"mm{cb}_{bt}", tag="mm")
                for kt in range(KT):
                    nc.tensor.matmul(
                        pmm[:],
                        v0T[kt][:, bt * 128:(bt + 1) * 128],
                        w_t[kt][:, cb * 512:(cb + 1) * 512],
                        start=(kt == 0), stop=(kt == KT - 1))
                sq = sqp.tile([128, 512], F32, name=f"sq{cb}_{bt}", tag="sq")
                nc.scalar.activation(sq[:], pmm[:],
                                     mybir.ActivationFunctionType.Square)
                gm = redp.tile([128, 2], F32, name=f"gm{cb}_{bt}", tag="gm")
                sq3 = sq[:].rearrange("p (g j) -> p g j", j=PATCH)
                nc.vector.tensor_reduce(gm[:], sq3, axis=mybir.AxisListType.X,
                                        op=mybir.AluOpType.max)
                rc = redp.tile([128, 2], F32, name=f"rc{cb}_{bt}", tag="rc")
                nc.vector.reciprocal(rc[:], gm[:])
                ot = outp.tile([128, 512], F32, name=f"ot{cb}_{bt}", tag="ot")
                nc.scalar.mul(ot[:, 0:PATCH], sq[:, 0:PATCH], rc[:, 0:1])
                nc.scalar.mul(ot[:, PATCH:512], sq[:, PATCH:512], rc[:, 1:2])
                nc.sync.dma_start(
                    out_d[bt * 128:(bt + 1) * 128, cb * 512:(cb + 1) * 512],
                    ot[:])


_CACHE = {}


def kernel(x: np.ndarray, q_params: np.ndarray) -> np.ndarray:
    x = np.ascontiguousarray(np.asarray(x, dtype=np.float32))
    q_params = np.asarray(q_params, dtype=np.float32)
    assert x.shape == (B, N_QUBITS) and q_params.shape == (G, Q_DEPTH * N_QUBITS)

    W = _build_W(q_params)
    ident = np.eye(128, dtype=np.float32)

    if "nc" not in _CACHE:
        _CACHE["nc"] = _build_nc()
    nc = _CACHE["nc"]

    in_maps = []
    for c in range(N_CORES):
        in_maps.append({
            "x": x[c * B_LOC:(c + 1) * B_LOC],
            "w": W,
            "ident": ident,
        })
    trace = bool(int(os.environ.get("BASS_KERNEL_TRACE", "0")))
    res = bass_utils.run_bass_kernel_spmd(nc, in_maps,
                                          core_ids=list(range(N_CORES)),
                                          trace=trace)
    _CACHE["last_result"] = res
    out = np.concatenate([res.results[c]["out"] for c in range(N_CORES)],
                         axis=0)
    return out.astype(np.float32)


if __name__ == "__main__":
    xs = np.load("/root/problem/work/x.npy")
    qs = np.load("/root/problem/work/qp.npy")
    o = kernel(xs, qs)
    print("out", o.shape, o.dtype)
    exp = np.load("/root/problem/work/expected_np.npy")
    rel = np.linalg.norm(o - exp) / np.linalg.norm(exp)
    print("rel l2 err vs numpy-replica expected:", rel)
    print("max abs err:", np.abs(o - exp).max())


# revision 6
# speedup vs baseline: 1.2318x; 1.2318x over previous
"""Patch-QGAN quantum generator kernel for Trainium2 (8 NeuronCores, SPMD).

Math: the reference evolves |0..0> through an RY embedding layer (angles x/2),
then Q_DEPTH=6 blocks of [per-generator RY layer + CZ-chain sign flip], then
returns probs[..., :256] / sum(probs) normalized by its per-(b,g) max.

Two structural facts make this a matmul problem:
  1. All gates are real; the embedding produces a rank-1 Kronecker vector
     v0(b) = kron_w [cos(x_bw/2), sin(x_bw/2)]  (wire 0 = MSB).
     The remaining 6 blocks depend only on q_params, so they fold into a
     fixed orthogonal matrix M_g per generator: state(b,g) = M_g @ v0(b).
  2. The sum-normalization cancels against the max-normalization:
     (p/S)/max(p/S) == p/max(p).  So only rows 0..255 of M_g are needed.

Per core (batch sharded 8 ways, 512 rows each):
  x -> cos/sin (ScalarE Sin LUT) -> Kronecker doubling (VectorE) -> v0
  -> PE transpose -> v0T -> fp32 matmul vs W = [M_g[:256,:].T]_g (1024x4096)
  -> square (ScalarE) -> per-256-chunk max + reciprocal (VectorE)
  -> scale (ScalarE) -> DMA out.
W is precomputed on host from q_params (tiny: 16x60) in float64.
"""

import os
import sys
import tempfile

import numpy as np

sys.path.insert(0, "/opt/trn_rl_repo")

import concourse.bass as bass
import concourse.tile as tile
from concourse import bacc, mybir
from concourse import bass_utils

N_QUBITS = 10
DIM = 1 << N_QUBITS           # 1024
PATCH = 256
G = 16
Q_DEPTH = 6
B = 4096
N_CORES = 8
B_LOC = B // N_CORES          # 512
BT = B_LOC // 128             # 4 batch tiles per core
KT = DIM // 128               # 8 contraction tiles
CB = (G * PATCH) // 512       # 8 column blocks of 512 (= 2 generators each)

F32 = mybir.dt.float32
F32R = mybir.dt.float32r


def _cz_sign():
    idx = np.arange(DIM)
    shifts = np.arange(N_QUBITS - 1, -1, -1)
    bits = (idx[:, None] >> shifts[None, :]) & 1
    pairs = bits[:, :-1] & bits[:, 1:]
    return np.where(pairs.sum(-1) % 2 == 1, -1.0, 1.0)


def _build_W(q_params: np.ndarray) -> np.ndarray:
    """Rows 0..255 of M_g = D K_5 D K_4 ... D K_0, stacked as (1024, G*256).

    Computed by right-multiplying basis rows S = I[:256] through the chain:
    S @ D scales columns by the CZ sign; S @ K_d applies kron_w RY(-theta_w)
    to each row (RY(t)^T = RY(-t))."""
    w = q_params.reshape(G, Q_DEPTH, N_QUBITS).astype(np.float64)
    sign = _cz_sign()
    S = np.zeros((G, PATCH, DIM))
    S[:, np.arange(PATCH), np.arange(PATCH)] = 1.0
    for d in range(Q_DEPTH - 1, -1, -1):
        S = S * sign[None, None, :]
        ang = -w[:, d, :]
        for wi in range(N_QUBITS):
            half = ang[:, wi] * 0.5
            c = np.cos(half)[:, None, None, None]
            s = np.sin(half)[:, None, None, None]
            St = S.reshape(G, PATCH, 1 << wi, 2, 1 << (N_QUBITS - wi - 1))
            s0, s1 = St[:, :, :, 0, :], St[:, :, :, 1, :]
            S = np.stack((c * s0 - s * s1, s * s0 + c * s1), axis=3)
            S = S.reshape(G, PATCH, DIM)
    # (G, 256, 1024) -> (1024, G*256), column index = g*256 + j
    Wall = np.ascontiguousarray(S.transpose(2, 0, 1).reshape(DIM, G * PATCH))
    return Wall.astype(np.float32)


def _build_nc():
    nc = bacc.Bacc("TRN2", target_bir_lowering=False, debug=False,
                   num_devices=N_CORES)
    x_d = nc.dram_tensor("x", [B_LOC, N_QUBITS], F32, kind="ExternalInput").ap()
    w_d = nc.dram_tensor("w", [DIM, G * PATCH], F32, kind="ExternalInput").ap()
    id_d = nc.dram_tensor("ident", [128, 128], F32, kind="ExternalInput").ap()
    out_d = nc.dram_tensor("out", [B_LOC, G * PATCH], F32,
                           kind="ExternalOutput").ap()

    with tile.TileContext(nc) as tc:
        _body(nc, tc, x_d, w_d, id_d, out_d)
    nc.compile()
    return nc


def _body(nc, tc, x_d, w_d, id_d, out_d):
    from contextlib import ExitStack
    ctx = ExitStack()
    with ctx:
        const = ctx.enter_context(tc.tile_pool(name="const", bufs=1))
        vpool = ctx.enter_context(tc.tile_pool(name="vpool", bufs=2))
        outp = ctx.enter_context(tc.tile_pool(name="outp", bufs=6))
        redp = ctx.enter_context(tc.tile_pool(name="redp", bufs=8))
        psmm = ctx.enter_context(tc.tile_pool(name="psmm", bufs=5, space="PSUM"))
        pstr = ctx.enter_context(tc.tile_pool(name="pstr", bufs=1, space="PSUM"))

        # Small inputs first: x feeds the whole v0 pipeline and must not
        # queue behind the 16.8MB weight load.
        x_t = []
        for bt in range(BT):
            xt = vpool.tile([128, N_QUBITS], F32, name=f"x{bt}", tag=f"x{bt}")
            nc.sync.dma_start(xt[:], x_d[bt * 128:(bt + 1) * 128, :])
            x_t.append(xt)
        ident = const.tile([128, 128], F32, name="ident_t")
        nc.sync.dma_start(ident[:], id_d[:, :])
        halfpi = const.tile([128, 1], F32, name="halfpi")
        nc.gpsimd.memset(halfpi[:], float(np.float32(np.pi / 2)))

        # Replicated weights: 8 k-tiles of [128, 4096], loaded column-block
        # major so matmuls on early column blocks can start before the whole
        # matrix has arrived.
        w_t = [const.tile([128, G * PATCH], F32R, name=f"w{kt}")
               for kt in range(KT)]
        for cb in range(CB):
            for kt in range(KT):
                nc.sync.dma_start(
                    w_t[kt][:, cb * 512:(cb + 1) * 512],
                    w_d[kt * 128:(kt + 1) * 128,
                        cb * 512:(cb + 1) * 512].bitcast(F32R))

        # v0T: [k, b] layout, 8 tiles of [128, 512]
        v0T = [const.tile([128, B_LOC], F32R, name=f"v0T{kt}")
               for kt in range(KT)]

        for bt in range(BT):
            # cs interleaved: col 2w = cos(x_w/2), col 2w+1 = sin(x_w/2)
            cs = vpool.tile([128, 2 * N_QUBITS], F32, name=f"cs{bt}", tag="cs")
            cs3 = cs[:].rearrange("p (w t) -> p w t", t=2)
            nc.scalar.activation(cs3[:, :, 1], x_t[bt][:],
                                 mybir.ActivationFunctionType.Sin,
                                 bias=0.0, scale=0.5)
            nc.scalar.activation(cs3[:, :, 0], x_t[bt][:],
                                 mybir.ActivationFunctionType.Sin,
                                 bias=halfpi[:], scale=0.5)

            # Kronecker doubling, wire 9 innermost first; one tensor_tensor
            # per level: out[:, m*L + t] = cur[:, t] * cs[:, 2w + m]
            # via broadcast access patterns (in0 repeats the L block twice,
            # in1 holds each of cos/sin constant across L).
            vA = vpool.tile([128, DIM], F32, name=f"vA{bt}", tag="vA")
            vB = vpool.tile([128, DIM], F32, name=f"vB{bt}", tag="vB")
            nc.vector.tensor_copy(vA[:, 0:2], cs3[:, 9, :])
            cur, nxt = vA, vB
            L = 2
            for wi in range(N_QUBITS - 2, -1, -1):
                in0 = cur[:, 0:L].rearrange("p (o l) -> p o l",
                                            o=1).broadcast_to((128, 2, L))
                in1 = cs[:, 2 * wi:2 * wi + 2].rearrange(
                    "p (m o) -> p m o", o=1).broadcast_to((128, 2, L))
                out3 = nxt[:, 0:2 * L].rearrange("p (m l) -> p m l", l=L)
                nc.vector.tensor_tensor(out3, in0, in1, mybir.AluOpType.mult)
                cur, nxt = nxt, cur
                L *= 2
            assert L == DIM

            # Transpose [b,k] -> [k,b] via PE, one 128x128 block per k-tile.
            for kt in range(KT):
                trp = pstr.tile([128, 128], F32, name=f"tr{bt}_{kt}", tag="tr", bufs=2)
                nc.tensor.transpose(trp[:], cur[:, kt * 128:(kt + 1) * 128],
                                    ident[:])
                # fp32 PSUM -> float32r SBUF (verifier requires fp32r-typed
                # producers for fp32r matmul operands; DVE does the cast)
                nc.vector.tensor_copy(
                    v0T[kt][:, bt * 128:(bt + 1) * 128], trp[:])

        # PE warmup: HAM un-throttles after ~3.4us of sustained matmul
        # activity; burn idle PE time during the v0 build so the real
        # matmuls run at 2.4GHz. Results are never read.
        pwarm = pstr.tile([128, 128], F32, name="pwarm", tag="warm")
        for i in range(24):
            nc.tensor.matmul(pwarm[:], ident[:], ident[:],
                             start=True, stop=True, skip_group_check=True)

        # Main pipeline per (cb, bt): 8 accumulating matmuls; then
        # rs = 1/max|state| (abs-max straight off PSUM), and a single
        # Square-with-scale pass: (state*rs)^2 == state^2 / max(state^2).
        for cb in range(CB):
            for bt in range(BT):
                pmm = psmm.tile([128, 512], F32, name=f"mm{cb}_{bt}", tag="mm")
                for kt in range(KT):
                    nc.tensor.matmul(
                        pmm[:],
                        v0T[kt][:, bt * 128:(bt + 1) * 128],
                        w_t[kt][:, cb * 512:(cb + 1) * 512],
                        start=(kt == 0), stop=(kt == KT - 1))
                gm = redp.tile([128, 2], F32, name=f"gm{cb}_{bt}", tag="gm")
                pm3 = pmm[:].rearrange("p (g j) -> p g j", j=PATCH)
                nc.vector.tensor_reduce(gm[:], pm3, axis=mybir.AxisListType.X,
                                        op=mybir.AluOpType.max,
                                        apply_absolute_value=True)
                rc = redp.tile([128, 2], F32, name=f"rc{cb}_{bt}", tag="rc")
                nc.vector.reciprocal(rc[:], gm[:])
                ot = outp.tile([128, 512], F32, name=f"ot{cb}_{bt}", tag="ot")
                nc.scalar.activation(ot[:, 0:PATCH], pmm[:, 0:PATCH],
                                     mybir.ActivationFunctionType.Square,
                                     scale=rc[:, 0:1])
                nc.scalar.activation(ot[:, PATCH:512], pmm[:, PATCH:512],
                                     mybir.ActivationFunctionType.Square,
                                     scale=rc[:, 1:2])
                nc.sync.dma_start(
                    out_d[bt * 128:(bt + 1) * 128, cb * 512:(cb + 1) * 512],
                    ot[:])


_CACHE = {}


def kernel(x: np.ndarray, q_params: np.ndarray) -> np.ndarray:
    x = np.ascontiguousarray(np.asarray(x, dtype=np.float32))
    q_params = np.asarray(q_params, dtype=np.float32)
    assert x.shape == (B, N_QUBITS) and q_params.shape == (G, Q_DEPTH * N_QUBITS)

    W = _build_W(q_params)
    ident = np.eye(128, dtype=np.float32)

    if "nc" not in _CACHE:
        _CACHE["nc"] = _build_nc()
    nc = _CACHE["nc"]

    in_maps = []
    for c in range(N_CORES):
        in_maps.append({
            "x": x[c * B_LOC:(c + 1) * B_LOC],
            "w": W,
            "ident": ident,
        })
    trace = bool(int(os.environ.get("BASS_KERNEL_TRACE", "0")))
    res = bass_utils.run_bass_kernel_spmd(nc, in_maps,
                                          core_ids=list(range(N_CORES)),
                                          trace=trace)
    _CACHE["last_result"] = res
    out = np.concatenate([res.results[c]["out"] for c in range(N_CORES)],
                         axis=0)
    return out.astype(np.float32)


if __name__ == "__main__":
    xs = np.load("/root/problem/work/x.npy")
    qs = np.load("/root/problem/work/qp.npy")
    o = kernel(xs, qs)
    print("out", o.shape, o.dtype)
    exp = np.load("/root/problem/work/expected_np.npy")
    rel = np.linalg.norm(o - exp) / np.linalg.norm(exp)
    print("rel l2 err vs numpy-replica expected:", rel)
    print("max abs err:", np.abs(o - exp).max())


# revision 7
# speedup vs baseline: 1.4402x; 1.1692x over previous
"""Patch-QGAN quantum generator kernel for Trainium2 (8 NeuronCores, SPMD).

Math: the reference evolves |0..0> through an RY embedding layer (angles x/2),
then Q_DEPTH=6 blocks of [per-generator RY layer + CZ-chain sign flip], then
returns probs[..., :256] / sum(probs) normalized by its per-(b,g) max.

Two structural facts make this a matmul problem:
  1. All gates are real; the embedding produces a rank-1 Kronecker vector
     v0(b) = kron_w [cos(x_bw/2), sin(x_bw/2)]  (wire 0 = MSB).
     The remaining 6 blocks depend only on q_params, so they fold into a
     fixed orthogonal matrix M_g per generator: state(b,g) = M_g @ v0(b).
  2. The sum-normalization cancels against the max-normalization:
     (p/S)/max(p/S) == p/max(p).  So only rows 0..255 of M_g are needed.

Per core (batch sharded 8 ways, 512 rows each):
  x -> cos/sin (ScalarE Sin LUT) -> Kronecker doubling (VectorE) -> v0
  -> PE transpose -> v0T -> fp32 matmul vs W = [M_g[:256,:].T]_g (1024x4096)
  -> square (ScalarE) -> per-256-chunk max + reciprocal (VectorE)
  -> scale (ScalarE) -> DMA out.
W is precomputed on host from q_params (tiny: 16x60) in float64.
"""

import os
import sys
import tempfile

import numpy as np

sys.path.insert(0, "/opt/trn_rl_repo")

import concourse.bass as bass
import concourse.tile as tile
from concourse import bacc, mybir
from concourse import bass_utils

N_QUBITS = 10
DIM = 1 << N_QUBITS           # 1024
PATCH = 256
G = 16
Q_DEPTH = 6
B = 4096
N_CORES = 8
B_LOC = B // N_CORES          # 512
BT = B_LOC // 128             # 4 batch tiles per core
KT = DIM // 128               # 8 contraction tiles
CB = (G * PATCH) // 512       # 8 column blocks of 512 (= 2 generators each)

F32 = mybir.dt.float32
F32R = mybir.dt.float32r


def _cz_sign():
    idx = np.arange(DIM)
    shifts = np.arange(N_QUBITS - 1, -1, -1)
    bits = (idx[:, None] >> shifts[None, :]) & 1
    pairs = bits[:, :-1] & bits[:, 1:]
    return np.where(pairs.sum(-1) % 2 == 1, -1.0, 1.0)


def _build_W(q_params: np.ndarray) -> np.ndarray:
    """Rows 0..255 of M_g = D K_5 D K_4 ... D K_0, stacked as (1024, G*256).

    Computed by right-multiplying basis rows S = I[:256] through the chain:
    S @ D scales columns by the CZ sign; S @ K_d applies kron_w RY(-theta_w)
    to each row (RY(t)^T = RY(-t))."""
    w = q_params.reshape(G, Q_DEPTH, N_QUBITS).astype(np.float64)
    sign = _cz_sign()
    S = np.zeros((G, PATCH, DIM))
    S[:, np.arange(PATCH), np.arange(PATCH)] = 1.0
    for d in range(Q_DEPTH - 1, -1, -1):
        S = S * sign[None, None, :]
        ang = -w[:, d, :]
        for wi in range(N_QUBITS):
            half = ang[:, wi] * 0.5
            c = np.cos(half)[:, None, None, None]
            s = np.sin(half)[:, None, None, None]
            St = S.reshape(G, PATCH, 1 << wi, 2, 1 << (N_QUBITS - wi - 1))
            s0, s1 = St[:, :, :, 0, :], St[:, :, :, 1, :]
            S = np.stack((c * s0 - s * s1, s * s0 + c * s1), axis=3)
            S = S.reshape(G, PATCH, DIM)
    # (G, 256, 1024) -> (1024, G*256), column index = g*256 + j
    Wall = np.ascontiguousarray(S.transpose(2, 0, 1).reshape(DIM, G * PATCH))
    return Wall.astype(np.float32)


def _build_nc():
    nc = bacc.Bacc("TRN2", target_bir_lowering=False, debug=False,
                   num_devices=N_CORES)
    x_d = nc.dram_tensor("x", [B_LOC, N_QUBITS], F32, kind="ExternalInput").ap()
    w_d = nc.dram_tensor("w", [DIM, G * PATCH], F32, kind="ExternalInput").ap()
    id_d = nc.dram_tensor("ident", [128, 128], F32, kind="ExternalInput").ap()
    out_d = nc.dram_tensor("out", [B_LOC, G * PATCH], F32,
                           kind="ExternalOutput").ap()

    with tile.TileContext(nc) as tc:
        _body(nc, tc, x_d, w_d, id_d, out_d)
    nc.compile()
    return nc


def _body(nc, tc, x_d, w_d, id_d, out_d):
    from contextlib import ExitStack
    ctx = ExitStack()
    with ctx:
        const = ctx.enter_context(tc.tile_pool(name="const", bufs=1))
        vpool = ctx.enter_context(tc.tile_pool(name="vpool", bufs=2))
        outp = ctx.enter_context(tc.tile_pool(name="outp", bufs=6))
        redp = ctx.enter_context(tc.tile_pool(name="redp", bufs=8))
        psmm = ctx.enter_context(tc.tile_pool(name="psmm", bufs=5, space="PSUM"))
        pstr = ctx.enter_context(tc.tile_pool(name="pstr", bufs=1, space="PSUM"))

        # Small inputs first: x feeds the whole v0 pipeline and must not
        # queue behind the 16.8MB weight load.
        x_t = []
        for bt in range(BT):
            xt = vpool.tile([128, N_QUBITS], F32, name=f"x{bt}", tag=f"x{bt}")
            nc.sync.dma_start(xt[:], x_d[bt * 128:(bt + 1) * 128, :])
            x_t.append(xt)
        ident = const.tile([128, 128], F32, name="ident_t")
        nc.sync.dma_start(ident[:], id_d[:, :])
        halfpi = const.tile([128, 1], F32, name="halfpi")
        nc.gpsimd.memset(halfpi[:], float(np.float32(np.pi / 2)))

        # Replicated weights: 8 k-tiles of [128, 4096], loaded column-block
        # major so matmuls on early column blocks can start before the whole
        # matrix has arrived.
        w_t = [const.tile([128, G * PATCH], F32R, name=f"w{kt}")
               for kt in range(KT)]
        for cb in range(CB):
            for kt in range(KT):
                nc.sync.dma_start(
                    w_t[kt][:, cb * 512:(cb + 1) * 512],
                    w_d[kt * 128:(kt + 1) * 128,
                        cb * 512:(cb + 1) * 512].bitcast(F32R))

        # v0T: [k, b] layout, 8 tiles of [128, 512]
        v0T = [const.tile([128, B_LOC], F32R, name=f"v0T{kt}")
               for kt in range(KT)]

        # PE warmup: HAM un-throttles after ~3.4us of sustained matmul
        # activity; burn idle PE time during the v0 build so the real
        # matmuls run at 2.4GHz. Results are never read.
        pwarm = pstr.tile([128, 128], F32, name="pwarm", tag="warm")
        for i in range(36):
            nc.tensor.matmul(pwarm[:], ident[:], ident[:],
                             start=True, stop=True, skip_group_check=True)

        for bt in range(BT):
            # cs interleaved: col 2w = cos(x_w/2), col 2w+1 = sin(x_w/2)
            cs = vpool.tile([128, 2 * N_QUBITS], F32, name=f"cs{bt}", tag="cs")
            cs3 = cs[:].rearrange("p (w t) -> p w t", t=2)
            nc.scalar.activation(cs3[:, :, 1], x_t[bt][:],
                                 mybir.ActivationFunctionType.Sin,
                                 bias=0.0, scale=0.5)
            nc.scalar.activation(cs3[:, :, 0], x_t[bt][:],
                                 mybir.ActivationFunctionType.Sin,
                                 bias=halfpi[:], scale=0.5)

            # Kronecker doubling, wire 9 innermost first; one tensor_tensor
            # per level: out[:, m*L + t] = cur[:, t] * cs[:, 2w + m]
            # via broadcast access patterns (in0 repeats the L block twice,
            # in1 holds each of cos/sin constant across L).
            vA = vpool.tile([128, DIM], F32, name=f"vA{bt}", tag="vA")
            vB = vpool.tile([128, DIM], F32, name=f"vB{bt}", tag="vB")
            nc.vector.tensor_copy(vA[:, 0:2], cs3[:, 9, :])
            cur, nxt = vA, vB
            L = 2
            for wi in range(N_QUBITS - 2, -1, -1):
                in0 = cur[:, 0:L].rearrange("p (o l) -> p o l",
                                            o=1).broadcast_to((128, 2, L))
                in1 = cs[:, 2 * wi:2 * wi + 2].rearrange(
                    "p (m o) -> p m o", o=1).broadcast_to((128, 2, L))
                out3 = nxt[:, 0:2 * L].rearrange("p (m l) -> p m l", l=L)
                nc.vector.tensor_tensor(out3, in0, in1, mybir.AluOpType.mult)
                cur, nxt = nxt, cur
                L *= 2
            assert L == DIM

            # Transpose [b,k] -> [k,b] via PE, one 128x128 block per k-tile.
            for kt in range(KT):
                trp = pstr.tile([128, 128], F32, name=f"tr{bt}_{kt}", tag="tr", bufs=2)
                nc.tensor.transpose(trp[:], cur[:, kt * 128:(kt + 1) * 128],
                                    ident[:])
                # fp32 PSUM -> float32r SBUF (verifier requires fp32r-typed
                # producers for fp32r matmul operands; ACT does the cast --
                # DVE is busier during the prologue)
                nc.scalar.copy(
                    v0T[kt][:, bt * 128:(bt + 1) * 128], trp[:])

        # Main pipeline per (cb, bt): 8 accumulating matmuls; then
        # rs = 1/max|state| (abs-max straight off PSUM), and a single
        # Square-with-scale pass: (state*rs)^2 == state^2 / max(state^2).
        for cb in range(CB):
            for bt in range(BT):
                pmm = psmm.tile([128, 512], F32, name=f"mm{cb}_{bt}", tag="mm")
                for kt in range(KT):
                    nc.tensor.matmul(
                        pmm[:],
                        v0T[kt][:, bt * 128:(bt + 1) * 128],
                        w_t[kt][:, cb * 512:(cb + 1) * 512],
                        start=(kt == 0), stop=(kt == KT - 1))
                gm = redp.tile([128, 2], F32, name=f"gm{cb}_{bt}", tag="gm")
                pm3 = pmm[:].rearrange("p (g j) -> p g j", j=PATCH)
                nc.vector.tensor_reduce(gm[:], pm3, axis=mybir.AxisListType.X,
                                        op=mybir.AluOpType.max,
                                        apply_absolute_value=True)
                rc = redp.tile([128, 2], F32, name=f"rc{cb}_{bt}", tag="rc")
                nc.vector.reciprocal(rc[:], gm[:])
                ot = outp.tile([128, 512], F32, name=f"ot{cb}_{bt}", tag="ot")
                nc.scalar.activation(ot[:, 0:PATCH], pmm[:, 0:PATCH],
                                     mybir.ActivationFunctionType.Square,
                                     scale=rc[:, 0:1])
                nc.scalar.activation(ot[:, PATCH:512], pmm[:, PATCH:512],
                                     mybir.ActivationFunctionType.Square,
                                     scale=rc[:, 1:2])
                nc.gpsimd.dma_start(
                    out_d[bt * 128:(bt + 1) * 128, cb * 512:(cb + 1) * 512],
                    ot[:])


_CACHE = {}


def kernel(x: np.ndarray, q_params: np.ndarray) -> np.ndarray:
    x = np.ascontiguousarray(np.asarray(x, dtype=np.float32))
    q_params = np.asarray(q_params, dtype=np.float32)
    assert x.shape == (B, N_QUBITS) and q_params.shape == (G, Q_DEPTH * N_QUBITS)

    W = _build_W(q_params)
    ident = np.eye(128, dtype=np.float32)

    if "nc" not in _CACHE:
        _CACHE["nc"] = _build_nc()
    nc = _CACHE["nc"]

    in_maps = []
    for c in range(N_CORES):
        in_maps.append({
            "x": x[c * B_LOC:(c + 1) * B_LOC],
            "w": W,
            "ident": ident,
        })
    trace = bool(int(os.environ.get("BASS_KERNEL_TRACE", "0")))
    res = bass_utils.run_bass_kernel_spmd(nc, in_maps,
                                          core_ids=list(range(N_CORES)),
                                          trace=trace)
    _CACHE["last_result"] = res
    out = np.concatenate([res.results[c]["out"] for c in range(N_CORES)],
                         axis=0)
    return out.astype(np.float32)


if __name__ == "__main__":
    xs = np.load("/root/problem/work/x.npy")
    qs = np.load("/root/problem/work/qp.npy")
    o = kernel(xs, qs)
    print("out", o.shape, o.dtype)
    exp = np.load("/root/problem/work/expected_np.npy")
    rel = np.linalg.norm(o - exp) / np.linalg.norm(exp)
    print("rel l2 err vs numpy-replica expected:", rel)
    print("max abs err:", np.abs(o - exp).max())


# revision 8
# speedup vs baseline: 1.5191x; 1.0548x over previous
"""Patch-QGAN quantum generator kernel for Trainium2 (8 NeuronCores, SPMD).

Math: the reference evolves |0..0> through an RY embedding layer (angles x/2),
then Q_DEPTH=6 blocks of [per-generator RY layer + CZ-chain sign flip], then
returns probs[..., :256] / sum(probs) normalized by its per-(b,g) max.

Two structural facts make this a matmul problem:
  1. All gates are real; the embedding produces a rank-1 Kronecker vector
     v0(b) = kron_w [cos(x_bw/2), sin(x_bw/2)]  (wire 0 = MSB).
     The remaining 6 blocks depend only on q_params, so they fold into a
     fixed orthogonal matrix M_g per generator: state(b,g) = M_g @ v0(b).
  2. The sum-normalization cancels against the max-normalization:
     (p/S)/max(p/S) == p/max(p).  So only rows 0..255 of M_g are needed.

Per core (batch sharded 8 ways, 512 rows each):
  x -> cos/sin (ScalarE Sin LUT) -> Kronecker doubling (VectorE) -> v0
  -> PE transpose -> v0T -> fp32 matmul vs W = [M_g[:256,:].T]_g (1024x4096)
  -> square (ScalarE) -> per-256-chunk max + reciprocal (VectorE)
  -> scale (ScalarE) -> DMA out.
W is precomputed on host from q_params (tiny: 16x60) in float64.
"""

import os
import sys
import tempfile

import numpy as np

sys.path.insert(0, "/opt/trn_rl_repo")

import concourse.bass as bass
import concourse.tile as tile
from concourse import bacc, mybir
from concourse import bass_utils

N_QUBITS = 10
DIM = 1 << N_QUBITS           # 1024
PATCH = 256
G = 16
Q_DEPTH = 6
B = 4096
N_CORES = 8
B_LOC = B // N_CORES          # 512
BT = B_LOC // 128             # 4 batch tiles per core
KT = DIM // 128               # 8 contraction tiles
CB = (G * PATCH) // 512       # 8 column blocks of 512 (= 2 generators each)

F32 = mybir.dt.float32
F32R = mybir.dt.float32r


def _cz_sign():
    idx = np.arange(DIM)
    shifts = np.arange(N_QUBITS - 1, -1, -1)
    bits = (idx[:, None] >> shifts[None, :]) & 1
    pairs = bits[:, :-1] & bits[:, 1:]
    return np.where(pairs.sum(-1) % 2 == 1, -1.0, 1.0)


def _build_W(q_params: np.ndarray) -> np.ndarray:
    """Rows 0..255 of M_g = D K_5 D K_4 ... D K_0, stacked as (1024, G*256).

    Computed by right-multiplying basis rows S = I[:256] through the chain:
    S @ D scales columns by the CZ sign; S @ K_d applies kron_w RY(-theta_w)
    to each row (RY(t)^T = RY(-t))."""
    w = q_params.reshape(G, Q_DEPTH, N_QUBITS).astype(np.float64)
    sign = _cz_sign()
    S = np.zeros((G, PATCH, DIM))
    S[:, np.arange(PATCH), np.arange(PATCH)] = 1.0
    for d in range(Q_DEPTH - 1, -1, -1):
        S = S * sign[None, None, :]
        ang = -w[:, d, :]
        for wi in range(N_QUBITS):
            half = ang[:, wi] * 0.5
            c = np.cos(half)[:, None, None, None]
            s = np.sin(half)[:, None, None, None]
            St = S.reshape(G, PATCH, 1 << wi, 2, 1 << (N_QUBITS - wi - 1))
            s0, s1 = St[:, :, :, 0, :], St[:, :, :, 1, :]
            S = np.stack((c * s0 - s * s1, s * s0 + c * s1), axis=3)
            S = S.reshape(G, PATCH, DIM)
    # (G, 256, 1024) -> (1024, G*256), column index = g*256 + j
    Wall = np.ascontiguousarray(S.transpose(2, 0, 1).reshape(DIM, G * PATCH))
    return Wall.astype(np.float32)


def _build_nc():
    nc = bacc.Bacc("TRN2", target_bir_lowering=False, debug=False,
                   num_devices=N_CORES)
    x_d = nc.dram_tensor("x", [B_LOC, N_QUBITS], F32, kind="ExternalInput").ap()
    w_d = nc.dram_tensor("w", [DIM, G * PATCH], F32, kind="ExternalInput").ap()
    id_d = nc.dram_tensor("ident", [128, 128], F32, kind="ExternalInput").ap()
    out_d = nc.dram_tensor("out", [B_LOC, G * PATCH], F32,
                           kind="ExternalOutput").ap()

    with tile.TileContext(nc) as tc:
        _body(nc, tc, x_d, w_d, id_d, out_d)
    nc.compile()
    return nc


def _body(nc, tc, x_d, w_d, id_d, out_d):
    from contextlib import ExitStack
    ctx = ExitStack()
    with ctx:
        const = ctx.enter_context(tc.tile_pool(name="const", bufs=1))
        vpool = ctx.enter_context(tc.tile_pool(name="vpool", bufs=2))
        outp = ctx.enter_context(tc.tile_pool(name="outp", bufs=6))
        redp = ctx.enter_context(tc.tile_pool(name="redp", bufs=8))
        psmm = ctx.enter_context(tc.tile_pool(name="psmm", bufs=5, space="PSUM"))
        pstr = ctx.enter_context(tc.tile_pool(name="pstr", bufs=1, space="PSUM"))

        # Small inputs first: ident gates the PE warmup, x feeds the v0
        # pipeline; neither may queue behind the 16.8MB weight load.
        ident = const.tile([128, 128], F32, name="ident_t")
        nc.sync.dma_start(ident[:], id_d[:, :])
        x_t = []
        for bt in range(BT):
            xt = vpool.tile([128, N_QUBITS], F32, name=f"x{bt}", tag=f"x{bt}")
            nc.sync.dma_start(xt[:], x_d[bt * 128:(bt + 1) * 128, :])
            x_t.append(xt)
        halfpi = const.tile([128, 1], F32, name="halfpi")
        nc.gpsimd.memset(halfpi[:], float(np.float32(np.pi / 2)))

        # Replicated weights: 8 k-tiles of [128, 4096], loaded column-block
        # major so matmuls on early column blocks can start before the whole
        # matrix has arrived.
        w_t = [const.tile([128, G * PATCH], F32R, name=f"w{kt}")
               for kt in range(KT)]
        for cb in range(CB):
            for kt in range(KT):
                nc.sync.dma_start(
                    w_t[kt][:, cb * 512:(cb + 1) * 512],
                    w_d[kt * 128:(kt + 1) * 128,
                        cb * 512:(cb + 1) * 512].bitcast(F32R))

        # v0T: [k, b] layout, 8 tiles of [128, 512]
        v0T = [const.tile([128, B_LOC], F32R, name=f"v0T{kt}")
               for kt in range(KT)]

        # PE warmup: HAM un-throttles after ~3.4us of sustained matmul
        # activity; burn idle PE time during the v0 build so the real
        # matmuls run at 2.4GHz. Results are never read.
        pwarm = pstr.tile([128, 128], F32, name="pwarm", tag="warm")
        for i in range(24):
            nc.tensor.matmul(pwarm[:], ident[:], ident[:],
                             start=True, stop=True, skip_group_check=True)

        for bt in range(BT):
            # cs interleaved: col 2w = cos(x_w/2), col 2w+1 = sin(x_w/2)
            cs = vpool.tile([128, 2 * N_QUBITS], F32, name=f"cs{bt}", tag="cs")
            cs3 = cs[:].rearrange("p (w t) -> p w t", t=2)
            nc.scalar.activation(cs3[:, :, 1], x_t[bt][:],
                                 mybir.ActivationFunctionType.Sin,
                                 bias=0.0, scale=0.5)
            nc.scalar.activation(cs3[:, :, 0], x_t[bt][:],
                                 mybir.ActivationFunctionType.Sin,
                                 bias=halfpi[:], scale=0.5)

            # Kronecker doubling, wire 9 innermost first; one tensor_tensor
            # per level: out[:, m*L + t] = cur[:, t] * cs[:, 2w + m]
            # via broadcast access patterns (in0 repeats the L block twice,
            # in1 holds each of cos/sin constant across L).
            vA = vpool.tile([128, DIM], F32, name=f"vA{bt}", tag="vA")
            vB = vpool.tile([128, DIM], F32, name=f"vB{bt}", tag="vB")
            nc.vector.tensor_copy(vA[:, 0:2], cs3[:, 9, :])
            cur, nxt = vA, vB
            L = 2
            for wi in range(N_QUBITS - 2, -1, -1):
                in0 = cur[:, 0:L].rearrange("p (o l) -> p o l",
                                            o=1).broadcast_to((128, 2, L))
                in1 = cs[:, 2 * wi:2 * wi + 2].rearrange(
                    "p (m o) -> p m o", o=1).broadcast_to((128, 2, L))
                out3 = nxt[:, 0:2 * L].rearrange("p (m l) -> p m l", l=L)
                nc.vector.tensor_tensor(out3, in0, in1, mybir.AluOpType.mult)
                cur, nxt = nxt, cur
                L *= 2
            assert L == DIM

            # Transpose [b,k] -> [k,b] via PE, one 128x128 block per k-tile.
            for kt in range(KT):
                trp = pstr.tile([128, 128], F32, name=f"tr{bt}_{kt}", tag="tr", bufs=2)
                nc.tensor.transpose(trp[:], cur[:, kt * 128:(kt + 1) * 128],
                                    ident[:])
                # fp32 PSUM -> float32r SBUF (verifier requires fp32r-typed
                # producers for fp32r matmul operands; ACT does the cast --
                # DVE is busier during the prologue)
                nc.scalar.copy(
                    v0T[kt][:, bt * 128:(bt + 1) * 128], trp[:])

        # Main pipeline per (cb, bt): 8 accumulating matmuls; then
        # rs = 1/max|state| (abs-max straight off PSUM), and a single
        # Square-with-scale pass: (state*rs)^2 == state^2 / max(state^2).
        for cb in range(CB):
            for bt in range(BT):
                pmm = psmm.tile([128, 512], F32, name=f"mm{cb}_{bt}", tag="mm")
                for kt in range(KT):
                    nc.tensor.matmul(
                        pmm[:],
                        v0T[kt][:, bt * 128:(bt + 1) * 128],
                        w_t[kt][:, cb * 512:(cb + 1) * 512],
                        start=(kt == 0), stop=(kt == KT - 1))
                gm = redp.tile([128, 2], F32, name=f"gm{cb}_{bt}", tag="gm")
                pm3 = pmm[:].rearrange("p (g j) -> p g j", j=PATCH)
                nc.vector.tensor_reduce(gm[:], pm3, axis=mybir.AxisListType.X,
                                        op=mybir.AluOpType.max,
                                        apply_absolute_value=True)
                rc = redp.tile([128, 2], F32, name=f"rc{cb}_{bt}", tag="rc")
                nc.vector.reciprocal(rc[:], gm[:])
                ot = outp.tile([128, 512], F32, name=f"ot{cb}_{bt}", tag="ot")
                nc.scalar.activation(ot[:, 0:PATCH], pmm[:, 0:PATCH],
                                     mybir.ActivationFunctionType.Square,
                                     scale=rc[:, 0:1])
                nc.scalar.activation(ot[:, PATCH:512], pmm[:, PATCH:512],
                                     mybir.ActivationFunctionType.Square,
                                     scale=rc[:, 1:2])
                nc.gpsimd.dma_start(
                    out_d[bt * 128:(bt + 1) * 128, cb * 512:(cb + 1) * 512],
                    ot[:])


_CACHE = {}


def kernel(x: np.ndarray, q_params: np.ndarray) -> np.ndarray:
    x = np.ascontiguousarray(np.asarray(x, dtype=np.float32))
    q_params = np.asarray(q_params, dtype=np.float32)
    assert x.shape == (B, N_QUBITS) and q_params.shape == (G, Q_DEPTH * N_QUBITS)

    W = _build_W(q_params)
    ident = np.eye(128, dtype=np.float32)

    if "nc" not in _CACHE:
        _CACHE["nc"] = _build_nc()
    nc = _CACHE["nc"]

    in_maps = []
    for c in range(N_CORES):
        in_maps.append({
            "x": x[c * B_LOC:(c + 1) * B_LOC],
            "w": W,
            "ident": ident,
        })
    trace = bool(int(os.environ.get("BASS_KERNEL_TRACE", "0")))
    res = bass_utils.run_bass_kernel_spmd(nc, in_maps,
                                          core_ids=list(range(N_CORES)),
                                          trace=trace)
    _CACHE["last_result"] = res
    out = np.concatenate([res.results[c]["out"] for c in range(N_CORES)],
                         axis=0)
    return out.astype(np.float32)


if __name__ == "__main__":
    xs = np.load("/root/problem/work/x.npy")
    qs = np.load("/root/problem/work/qp.npy")
    o = kernel(xs, qs)
    print("out", o.shape, o.dtype)
    exp = np.load("/root/problem/work/expected_np.npy")
    rel = np.linalg.norm(o - exp) / np.linalg.norm(exp)
    print("rel l2 err vs numpy-replica expected:", rel)
    print("max abs err:", np.abs(o - exp).max())


# revision 9
# speedup vs baseline: 1.5544x; 1.0232x over previous
"""Patch-QGAN quantum generator kernel for Trainium2 (8 NeuronCores, SPMD).

Math: the reference evolves |0..0> through an RY embedding layer (angles x/2),
then Q_DEPTH=6 blocks of [per-generator RY layer + CZ-chain sign flip], then
returns probs[..., :256] / sum(probs) normalized by its per-(b,g) max.

Two structural facts make this a matmul problem:
  1. All gates are real; the embedding produces a rank-1 Kronecker vector
     v0(b) = kron_w [cos(x_bw/2), sin(x_bw/2)]  (wire 0 = MSB).
     The remaining 6 blocks depend only on q_params, so they fold into a
     fixed orthogonal matrix M_g per generator: state(b,g) = M_g @ v0(b).
  2. The sum-normalization cancels against the max-normalization:
     (p/S)/max(p/S) == p/max(p).  So only rows 0..255 of M_g are needed.

Per core (batch sharded 8 ways, 512 rows each):
  x -> cos/sin (ScalarE Sin LUT) -> Kronecker doubling (VectorE) -> v0
  -> PE transpose -> v0T -> fp32 matmul vs W = [M_g[:256,:].T]_g (1024x4096)
  -> square (ScalarE) -> per-256-chunk max + reciprocal (VectorE)
  -> scale (ScalarE) -> DMA out.
W is precomputed on host from q_params (tiny: 16x60) in float64.
"""

import os
import sys
import tempfile

import numpy as np

sys.path.insert(0, "/opt/trn_rl_repo")

import concourse.bass as bass
import concourse.tile as tile
from concourse import bacc, mybir
from concourse import bass_utils

N_QUBITS = 10
DIM = 1 << N_QUBITS           # 1024
PATCH = 256
G = 16
Q_DEPTH = 6
B = 4096
N_CORES = 8
B_LOC = B // N_CORES          # 512
BT = B_LOC // 128             # 4 batch tiles per core
KT = DIM // 128               # 8 contraction tiles
CB = (G * PATCH) // 512       # 8 column blocks of 512 (= 2 generators each)

F32 = mybir.dt.float32
F32R = mybir.dt.float32r


def _cz_sign():
    idx = np.arange(DIM)
    shifts = np.arange(N_QUBITS - 1, -1, -1)
    bits = (idx[:, None] >> shifts[None, :]) & 1
    pairs = bits[:, :-1] & bits[:, 1:]
    return np.where(pairs.sum(-1) % 2 == 1, -1.0, 1.0)


def _build_W(q_params: np.ndarray) -> np.ndarray:
    """Rows 0..255 of M_g = D K_5 D K_4 ... D K_0, stacked as (1024, G*256).

    Computed by right-multiplying basis rows S = I[:256] through the chain:
    S @ D scales columns by the CZ sign; S @ K_d applies kron_w RY(-theta_w)
    to each row (RY(t)^T = RY(-t))."""
    w = q_params.reshape(G, Q_DEPTH, N_QUBITS).astype(np.float64)
    sign = _cz_sign()
    S = np.zeros((G, PATCH, DIM))
    S[:, np.arange(PATCH), np.arange(PATCH)] = 1.0
    for d in range(Q_DEPTH - 1, -1, -1):
        S = S * sign[None, None, :]
        ang = -w[:, d, :]
        for wi in range(N_QUBITS):
            half = ang[:, wi] * 0.5
            c = np.cos(half)[:, None, None, None]
            s = np.sin(half)[:, None, None, None]
            St = S.reshape(G, PATCH, 1 << wi, 2, 1 << (N_QUBITS - wi - 1))
            s0, s1 = St[:, :, :, 0, :], St[:, :, :, 1, :]
            S = np.stack((c * s0 - s * s1, s * s0 + c * s1), axis=3)
            S = S.reshape(G, PATCH, DIM)
    # (G, 256, 1024) -> (1024, G*256), column index = g*256 + j
    Wall = np.ascontiguousarray(S.transpose(2, 0, 1).reshape(DIM, G * PATCH))
    return Wall.astype(np.float32)


def _build_nc():
    nc = bacc.Bacc("TRN2", target_bir_lowering=False, debug=False,
                   num_devices=N_CORES)
    x_d = nc.dram_tensor("x", [B_LOC, N_QUBITS], F32, kind="ExternalInput").ap()
    w_d = nc.dram_tensor("w", [DIM, G * PATCH], F32, kind="ExternalInput").ap()
    id_d = nc.dram_tensor("ident", [128, 128], F32, kind="ExternalInput").ap()
    out_d = nc.dram_tensor("out", [B_LOC, G * PATCH], F32,
                           kind="ExternalOutput").ap()

    with tile.TileContext(nc) as tc:
        _body(nc, tc, x_d, w_d, id_d, out_d)
    nc.compile()
    return nc


def _body(nc, tc, x_d, w_d, id_d, out_d):
    from contextlib import ExitStack
    ctx = ExitStack()
    with ctx:
        const = ctx.enter_context(tc.tile_pool(name="const", bufs=1))
        vpool = ctx.enter_context(tc.tile_pool(name="vpool", bufs=2))
        outp = ctx.enter_context(tc.tile_pool(name="outp", bufs=6))
        redp = ctx.enter_context(tc.tile_pool(name="redp", bufs=8))
        psmm = ctx.enter_context(tc.tile_pool(name="psmm", bufs=5, space="PSUM"))
        pstr = ctx.enter_context(tc.tile_pool(name="pstr", bufs=1, space="PSUM"))

        # Small inputs first: ident gates the PE warmup, x feeds the v0
        # pipeline; neither may queue behind the 16.8MB weight load.
        ident = const.tile([128, 128], F32, name="ident_t")
        nc.sync.dma_start(ident[:], id_d[:, :])
        x_t = []
        for bt in range(BT):
            xt = vpool.tile([128, N_QUBITS], F32, name=f"x{bt}", tag=f"x{bt}")
            nc.sync.dma_start(xt[:], x_d[bt * 128:(bt + 1) * 128, :])
            x_t.append(xt)
        halfpi = const.tile([128, 1], F32, name="halfpi")
        nc.gpsimd.memset(halfpi[:], float(np.float32(np.pi / 2)))

        # Replicated weights: 8 k-tiles of [128, 4096], loaded column-block
        # major so matmuls on early column blocks can start before the whole
        # matrix has arrived.
        w_t = [const.tile([128, G * PATCH], F32R, name=f"w{kt}")
               for kt in range(KT)]
        for cb in range(CB):
            for kt in range(KT):
                nc.sync.dma_start(
                    w_t[kt][:, cb * 512:(cb + 1) * 512],
                    w_d[kt * 128:(kt + 1) * 128,
                        cb * 512:(cb + 1) * 512].bitcast(F32R))

        # v0T: [k, b] layout, 8 tiles of [128, 512]
        v0T = [const.tile([128, B_LOC], F32R, name=f"v0T{kt}")
               for kt in range(KT)]

        # PE warmup: HAM un-throttles after ~3.4us of sustained matmul
        # activity; burn idle PE time during the v0 build so the real
        # matmuls run at 2.4GHz. Results are never read.
        pwarm = pstr.tile([128, 128], F32, name="pwarm", tag="warm")
        for i in range(12):
            nc.tensor.matmul(pwarm[:], ident[:], ident[:],
                             start=True, stop=True, skip_group_check=True)

        for bt in range(BT):
            # cs interleaved: col 2w = cos(x_w/2), col 2w+1 = sin(x_w/2)
            cs = vpool.tile([128, 2 * N_QUBITS], F32, name=f"cs{bt}", tag="cs")
            cs3 = cs[:].rearrange("p (w t) -> p w t", t=2)
            nc.scalar.activation(cs3[:, :, 1], x_t[bt][:],
                                 mybir.ActivationFunctionType.Sin,
                                 bias=0.0, scale=0.5)
            nc.scalar.activation(cs3[:, :, 0], x_t[bt][:],
                                 mybir.ActivationFunctionType.Sin,
                                 bias=halfpi[:], scale=0.5)

            # Kronecker doubling, wire 9 innermost first; one tensor_tensor
            # per level: out[:, m*L + t] = cur[:, t] * cs[:, 2w + m]
            # via broadcast access patterns (in0 repeats the L block twice,
            # in1 holds each of cos/sin constant across L).
            vA = vpool.tile([128, DIM], F32, name=f"vA{bt}", tag="vA")
            vB = vpool.tile([128, DIM], F32, name=f"vB{bt}", tag="vB")
            nc.vector.tensor_copy(vA[:, 0:2], cs3[:, 9, :])
            cur, nxt = vA, vB
            L = 2
            for wi in range(N_QUBITS - 2, -1, -1):
                in0 = cur[:, 0:L].rearrange("p (o l) -> p o l",
                                            o=1).broadcast_to((128, 2, L))
                in1 = cs[:, 2 * wi:2 * wi + 2].rearrange(
                    "p (m o) -> p m o", o=1).broadcast_to((128, 2, L))
                out3 = nxt[:, 0:2 * L].rearrange("p (m l) -> p m l", l=L)
                nc.vector.tensor_tensor(out3, in0, in1, mybir.AluOpType.mult)
                cur, nxt = nxt, cur
                L *= 2
            assert L == DIM

            # Transpose [b,k] -> [k,b] via PE, one 128x128 block per k-tile.
            for kt in range(KT):
                trp = pstr.tile([128, 128], F32, name=f"tr{bt}_{kt}", tag="tr", bufs=2)
                nc.tensor.transpose(trp[:], cur[:, kt * 128:(kt + 1) * 128],
                                    ident[:])
                # fp32 PSUM -> float32r SBUF (verifier requires fp32r-typed
                # producers for fp32r matmul operands; ACT does the cast --
                # DVE is busier during the prologue)
                nc.scalar.copy(
                    v0T[kt][:, bt * 128:(bt + 1) * 128], trp[:])

        # Main pipeline per (cb, bt): 8 accumulating matmuls; then
        # rs = 1/max|state| (abs-max straight off PSUM), and a single
        # Square-with-scale pass: (state*rs)^2 == state^2 / max(state^2).
        for cb in range(CB):
            for bt in range(BT):
                pmm = psmm.tile([128, 512], F32, name=f"mm{cb}_{bt}", tag="mm")
                for kt in range(KT):
                    nc.tensor.matmul(
                        pmm[:],
                        v0T[kt][:, bt * 128:(bt + 1) * 128],
                        w_t[kt][:, cb * 512:(cb + 1) * 512],
                        start=(kt == 0), stop=(kt == KT - 1))
                gm = redp.tile([128, 2], F32, name=f"gm{cb}_{bt}", tag="gm")
                pm3 = pmm[:].rearrange("p (g j) -> p g j", j=PATCH)
                nc.vector.tensor_reduce(gm[:], pm3, axis=mybir.AxisListType.X,
                                        op=mybir.AluOpType.max,
                                        apply_absolute_value=True)
                rc = redp.tile([128, 2], F32, name=f"rc{cb}_{bt}", tag="rc")
                nc.vector.reciprocal(rc[:], gm[:])
                ot = outp.tile([128, 512], F32, name=f"ot{cb}_{bt}", tag="ot")
                nc.scalar.activation(ot[:, 0:PATCH], pmm[:, 0:PATCH],
                                     mybir.ActivationFunctionType.Square,
                                     scale=rc[:, 0:1])
                nc.scalar.activation(ot[:, PATCH:512], pmm[:, PATCH:512],
                                     mybir.ActivationFunctionType.Square,
                                     scale=rc[:, 1:2])
                dma_eng = nc.gpsimd if cb < 6 else nc.sync
                dma_eng.dma_start(
                    out_d[bt * 128:(bt + 1) * 128, cb * 512:(cb + 1) * 512],
                    ot[:])


_CACHE = {}


def kernel(x: np.ndarray, q_params: np.ndarray) -> np.ndarray:
    x = np.ascontiguousarray(np.asarray(x, dtype=np.float32))
    q_params = np.asarray(q_params, dtype=np.float32)
    assert x.shape == (B, N_QUBITS) and q_params.shape == (G, Q_DEPTH * N_QUBITS)

    W = _build_W(q_params)
    ident = np.eye(128, dtype=np.float32)

    if "nc" not in _CACHE:
        _CACHE["nc"] = _build_nc()
    nc = _CACHE["nc"]

    in_maps = []
    for c in range(N_CORES):
        in_maps.append({
            "x": x[c * B_LOC:(c + 1) * B_LOC],
            "w": W,
            "ident": ident,
        })
    trace = bool(int(os.environ.get("BASS_KERNEL_TRACE", "0")))
    res = bass_utils.run_bass_kernel_spmd(nc, in_maps,
                                          core_ids=list(range(N_CORES)),
                                          trace=trace)
    _CACHE["last_result"] = res
    out = np.concatenate([res.results[c]["out"] for c in range(N_CORES)],
                         axis=0)
    return out.astype(np.float32)


if __name__ == "__main__":
    xs = np.load("/root/problem/work/x.npy")
    qs = np.load("/root/problem/work/qp.npy")
    o = kernel(xs, qs)
    print("out", o.shape, o.dtype)
    exp = np.load("/root/problem/work/expected_np.npy")
    rel = np.linalg.norm(o - exp) / np.linalg.norm(exp)
    print("rel l2 err vs numpy-replica expected:", rel)
    print("max abs err:", np.abs(o - exp).max())
